# revision 2
# baseline (speedup 1.0000x reference)
"""SVGD actor sampler (nn_ActorSvgd) on 8 trn2 NeuronCores.

Strategy: pure data parallel over the env-batch dimension B (8192 rows of
NP=20 particles). Particles only interact within a batch row, so we:
  * gather a0 = a0_table[idx] on host (pure indexing),
  * shard obs/a0 on B across the 8 cores (1024 batch rows / 20480 particle
    rows per core), replicate the small Q-network weights,
  * run the SVGD loop with a host-driven loop of pmap'd single-step updates
    (one compiled step graph, executed 10x on all 8 cores; no collectives),
  * concatenate per-core outputs back to full shape.
Falls back to single-device/CPU execution if the 8-core path is unavailable.
"""

import numpy as np

B = 8192
NP = 20
OBS = 17
ACT = 6
H = 256
N = B * NP
ACT_LIMIT = 1.0
SIGMA_P0 = 0.1
STEPS = 10
LR = 0.05
GAMMA = 1.0 / (1e-8 + 2.0 * 2.0**2)  # kernel_sigma = 2.0
M_CORES = 8

_cached = {}


def _make_fns(B_loc):
    import jax
    import jax.numpy as jnp

    def _q_net(x, W1, b1, W2, b2, W3, b3):
        h = jax.nn.relu(x @ W1 + b1)
        h = jax.nn.relu(h @ W2 + b2)
        return (h @ W3 + b3)[:, 0]

    N_loc = B_loc * NP

    def step(a, logp, obs, q1_W1, q1_b1, q1_W2, q1_b2, q1_W3, q1_b3,
             q2_W1, q2_b1, q2_W2, q2_b2, q2_W3, q2_b3):
        eye = jnp.eye(NP, dtype=obs.dtype)

        def q_min(X):
            xi = jnp.concatenate([obs, X], axis=-1)
            return jnp.minimum(
                _q_net(xi, q1_W1, q1_b1, q1_W2, q1_b2, q1_W3, q1_b3),
                _q_net(xi, q2_W1, q2_b1, q2_W2, q2_b2, q2_W3, q2_b3))

        logq, vjp_fn = jax.vjp(q_min, a)
        score = vjp_fn(jnp.ones_like(logq))[0]
        X = a.reshape(B_loc, NP, ACT)
        S = score.reshape(B_loc, NP, ACT)
        diff = X[:, :, None, :] - X[:, None, :, :]
        dist_sq = (diff**2).sum(-1)
        K = jnp.exp(-GAMMA * dist_sq)
        K_grad = -2.0 * GAMMA * diff * K[..., None]
        phi = (jnp.einsum('bij,bjd->bid', K, S) + K_grad.sum(1)) / NP
        term1 = (K_grad * S[:, None, :, :]).sum(-1).mean(2)
        term2 = -2.0 * GAMMA * ((K_grad.transpose(0, 2, 1, 3) * diff).sum(-1)
                                - NP * (K - eye)).mean(1)
        logp = logp - LR * (term1 + term2)
        a = a + LR * phi.reshape(N_loc, ACT)
        a = jnp.clip(a, -ACT_LIMIT, ACT_LIMIT)
        return a, logp

    def finalize(a, logp_svgd, a0):
        a = ACT_LIMIT * jnp.tanh(a)
        logp_normal = (-ACT * 0.5 * np.log(2.0 * np.pi * SIGMA_P0)
                       - 0.5 / SIGMA_P0 * (a0**2).sum(-1).reshape(B_loc, NP))
        logp_tanh = -(2.0 * (np.log(2.0) - a
                             - jax.nn.softplus(-2.0 * a))).sum(-1).reshape(
                                 B_loc, NP)
        logp_a = (logp_normal + logp_svgd + logp_tanh).mean(-1)
        return a, logp_a

    return step, finalize


def _run_pmap(obs_sh, a0_sh, weights):
    """Run data-parallel across 8 NeuronCores. obs_sh/a0_sh: [8, ...]."""
    import jax

    B_loc = B // M_CORES
    devs = jax.devices()[:M_CORES]
    if 'step_fn' not in _cached:
        step, finalize = _make_fns(B_loc)
        _cached['step_fn'] = jax.pmap(
            step, in_axes=(0, 0, 0) + (None,) * 12, devices=devs)
        _cached['fin_fn'] = jax.pmap(finalize, in_axes=(0, 0, 0), devices=devs)

    step_fn = _cached['step_fn']
    fin_fn = _cached['fin_fn']

    a = a0_sh
    logp = np.zeros((M_CORES, B_loc, NP), dtype=np.float32)
    for _ in range(STEPS):
        a, logp = step_fn(a, logp, obs_sh, *weights)
    a_sh, logp_sh = fin_fn(a, logp, a0_sh)
    a_out = np.asarray(a_sh).reshape(N, ACT)
    logp_out = np.asarray(logp_sh).reshape(B)
    return a_out, logp_out


def _run_host(obs, a0, weights):
    import jax

    if 'cpu_fns' not in _cached:
        with jax.default_device(jax.devices('cpu')[0]):
            step, finalize = _make_fns(B)
            _cached['cpu_fns'] = (jax.jit(step), jax.jit(finalize))
    step_fn, fin_fn = _cached['cpu_fns']
    import jax.numpy as jnp

    with jax.default_device(jax.devices('cpu')[0]):
        a = jnp.asarray(a0)
        logp = jnp.zeros((B, NP), dtype=jnp.float32)
        for _ in range(STEPS):
            a, logp = step_fn(a, logp, obs, *weights)
        a_out, logp_out = fin_fn(a, logp, a0)
    return (np.asarray(a_out, dtype=np.float32),
            np.asarray(logp_out, dtype=np.float32))


def kernel(obs, a0_table, q1_W1, q1_b1, q1_W2, q1_b2, q1_W3, q1_b3,
           q2_W1, q2_b1, q2_W2, q2_b2, q2_W3, q2_b3, idx):
    obs = np.asarray(obs, dtype=np.float32)
    a0 = np.ascontiguousarray(
        np.asarray(a0_table, dtype=np.float32)[np.asarray(idx)])
    weights = tuple(
        np.asarray(w, dtype=np.float32)
        for w in (q1_W1, q1_b1, q1_W2, q1_b2, q1_W3, q1_b3,
                  q2_W1, q2_b1, q2_W2, q2_b2, q2_W3, q2_b3))

    N_loc = N // M_CORES
    obs_sh = obs.reshape(M_CORES, N_loc, OBS)
    a0_sh = a0.reshape(M_CORES, N_loc, ACT)

    try:
        a, logp = _run_pmap(obs_sh, a0_sh, weights)
        if not (np.all(np.isfinite(a)) and np.all(np.isfinite(logp))):
            raise RuntimeError("non-finite device output")
    except Exception:
        _cached.pop('step_fn', None)
        _cached.pop('fin_fn', None)
        a, logp = _run_host(obs, a0, weights)

    return (np.asarray(a, dtype=np.float32).reshape(N, ACT),
            np.asarray(logp, dtype=np.float32).reshape(B))


# revision 3
# speedup vs baseline: 8.4627x; 8.4627x over previous
"""SVGD actor sampler (nn_ActorSvgd) on 8 trn2 NeuronCores.

Strategy: pure data parallel over the env-batch dimension B (8192 rows of
NP=20 particles). Particles only interact within a batch row, so we:
  * gather a0 = a0_table[idx] on host (pure indexing),
  * shard obs/a0 on B across the 8 cores (1024 batch rows / 20480 particle
    rows per core), replicate the small Q-network weights,
  * run the SVGD loop as a host-driven loop of pmap'd single-step updates
    (one compiled step graph, executed 10x on all 8 cores; no collectives),
  * finalize (tanh + log-prob assembly) on host — tiny, and neuronxcc's
    lower_act pass rejects the tanh+softplus combination,
  * concatenate per-core outputs back to full shape.
Falls back to single-device CPU execution if the 8-core path is unavailable.
"""

import numpy as np

B = 8192
NP = 20
OBS = 17
ACT = 6
H = 256
N = B * NP
ACT_LIMIT = 1.0
SIGMA_P0 = 0.1
STEPS = 10
LR = 0.05
GAMMA = 1.0 / (1e-8 + 2.0 * 2.0**2)  # kernel_sigma = 2.0
M_CORES = 8

_cached = {}


def _make_step(B_loc):
    import jax
    import jax.numpy as jnp

    def _q_net(x, W1, b1, W2, b2, W3, b3):
        h = jax.nn.relu(x @ W1 + b1)
        h = jax.nn.relu(h @ W2 + b2)
        return (h @ W3 + b3)[:, 0]

    N_loc = B_loc * NP

    def step(a, logp, obs, q1_W1, q1_b1, q1_W2, q1_b2, q1_W3, q1_b3,
             q2_W1, q2_b1, q2_W2, q2_b2, q2_W3, q2_b3):
        eye = jnp.eye(NP, dtype=obs.dtype)

        def q_min(X):
            xi = jnp.concatenate([obs, X], axis=-1)
            return jnp.minimum(
                _q_net(xi, q1_W1, q1_b1, q1_W2, q1_b2, q1_W3, q1_b3),
                _q_net(xi, q2_W1, q2_b1, q2_W2, q2_b2, q2_W3, q2_b3))

        logq, vjp_fn = jax.vjp(q_min, a)
        score = vjp_fn(jnp.ones_like(logq))[0]
        X = a.reshape(B_loc, NP, ACT)
        S = score.reshape(B_loc, NP, ACT)
        diff = X[:, :, None, :] - X[:, None, :, :]
        dist_sq = (diff**2).sum(-1)
        K = jnp.exp(-GAMMA * dist_sq)
        K_grad = -2.0 * GAMMA * diff * K[..., None]
        phi = (jnp.einsum('bij,bjd->bid', K, S) + K_grad.sum(1)) / NP
        term1 = (K_grad * S[:, None, :, :]).sum(-1).mean(2)
        term2 = -2.0 * GAMMA * ((K_grad.transpose(0, 2, 1, 3) * diff).sum(-1)
                                - NP * (K - eye)).mean(1)
        logp = logp - LR * (term1 + term2)
        a = a + LR * phi.reshape(N_loc, ACT)
        a = jnp.clip(a, -ACT_LIMIT, ACT_LIMIT)
        return a, logp

    return step


def _finalize_host(a_pre, logp_svgd, a0):
    """tanh squash + log-prob assembly, float32 numpy on host."""
    a_pre = np.asarray(a_pre, dtype=np.float32).reshape(N, ACT)
    logp_svgd = np.asarray(logp_svgd, dtype=np.float32).reshape(B, NP)
    a = (ACT_LIMIT * np.tanh(a_pre)).astype(np.float32)
    logp_normal = (-ACT * 0.5 * np.log(2.0 * np.pi * SIGMA_P0)
                   - 0.5 / SIGMA_P0 * (a0**2).sum(-1).reshape(B, NP))
    # softplus(-2a), numerically fine for |a| <= 1
    sp = np.log1p(np.exp(-2.0 * a))
    logp_tanh = -(2.0 * (np.log(2.0) - a - sp)).sum(-1).reshape(B, NP)
    logp_a = (logp_normal + logp_svgd.astype(np.float64)
              + logp_tanh).mean(-1)
    return a, logp_a.astype(np.float32)


def _run_pmap(obs_sh, a0_sh):
    """Run the 10 SVGD steps data-parallel across 8 NeuronCores."""
    import jax

    B_loc = B // M_CORES
    devs = jax.devices()[:M_CORES]
    if 'step_fn' not in _cached:
        _cached['step_fn'] = jax.pmap(
            _make_step(B_loc), in_axes=(0, 0, 0) + (None,) * 12,
            devices=devs)
    step_fn = _cached['step_fn']
    weights = _cached['weights']

    a = a0_sh
    logp = np.zeros((M_CORES, B_loc, NP), dtype=np.float32)
    for _ in range(STEPS):
        a, logp = step_fn(a, logp, obs_sh, *weights)
    a.block_until_ready()
    return np.asarray(a), np.asarray(logp)


def _run_host(obs, a0):
    import jax
    import jax.numpy as jnp

    cpu = jax.devices('cpu')[0]
    with jax.default_device(cpu):
        if 'cpu_fn' not in _cached:
            _cached['cpu_fn'] = jax.jit(_make_step(B))
        step_fn = _cached['cpu_fn']
        weights = _cached['weights']
        a = jnp.asarray(a0)
        logp = jnp.zeros((B, NP), dtype=jnp.float32)
        for _ in range(STEPS):
            a, logp = step_fn(a, logp, obs, *weights)
    return np.asarray(a), np.asarray(logp)


def kernel(obs, a0_table, q1_W1, q1_b1, q1_W2, q1_b2, q1_W3, q1_b3,
           q2_W1, q2_b1, q2_W2, q2_b2, q2_W3, q2_b3, idx):
    obs = np.asarray(obs, dtype=np.float32)
    a0 = np.ascontiguousarray(
        np.asarray(a0_table, dtype=np.float32)[np.asarray(idx)])
    _cached['weights'] = tuple(
        np.asarray(w, dtype=np.float32)
        for w in (q1_W1, q1_b1, q1_W2, q1_b2, q1_W3, q1_b3,
                  q2_W1, q2_b1, q2_W2, q2_b2, q2_W3, q2_b3))

    N_loc = N // M_CORES
    obs_sh = obs.reshape(M_CORES, N_loc, OBS)
    a0_sh = a0.reshape(M_CORES, N_loc, ACT)

    try:
        a_pre, logp_svgd = _run_pmap(obs_sh, a0_sh)
        if not (np.all(np.isfinite(a_pre))
                and np.all(np.isfinite(logp_svgd))):
            raise RuntimeError("non-finite device output")
    except Exception:
        _cached.pop('step_fn', None)
        a_pre, logp_svgd = _run_host(obs, a0)

    a, logp_a = _finalize_host(a_pre, logp_svgd, a0)
    return a.reshape(N, ACT), logp_a.reshape(B)


# revision 4
# speedup vs baseline: 32.2998x; 3.8167x over previous
"""SVGD actor sampler (nn_ActorSvgd) on 8 trn2 NeuronCores.

Strategy: pure data parallel over the env-batch dimension B (8192 rows of
NP=20 particles). Particles only interact within a batch row, so we:
  * gather a0 = a0_table[idx] on host (pure indexing),
  * shard obs/a0 on B across the 8 cores (1024 batch rows / 20480 particle
    rows per core), replicate the small Q-network weights,
  * run the SVGD loop as a host-driven loop of pmap'd single-step updates
    (one compiled step graph, executed 10x on all 8 cores; no collectives),
  * finalize (tanh + log-prob assembly) on host — tiny, and neuronxcc's
    lower_act pass rejects the tanh+softplus combination,
  * concatenate per-core outputs back to full shape.
Falls back to single-device CPU execution if the 8-core path is unavailable.
"""

import numpy as np

B = 8192
NP = 20
OBS = 17
ACT = 6
H = 256
N = B * NP
ACT_LIMIT = 1.0
SIGMA_P0 = 0.1
STEPS = 10
LR = 0.05
GAMMA = 1.0 / (1e-8 + 2.0 * 2.0**2)  # kernel_sigma = 2.0
M_CORES = 8

_cached = {}


def _make_step(B_loc):
    import jax
    import jax.numpy as jnp

    def _q_net(x, W1, b1, W2, b2, W3, b3):
        h = jax.nn.relu(x @ W1 + b1)
        h = jax.nn.relu(h @ W2 + b2)
        return (h @ W3 + b3)[:, 0]

    N_loc = B_loc * NP

    def step(a, logp, obs, q1_W1, q1_b1, q1_W2, q1_b2, q1_W3, q1_b3,
             q2_W1, q2_b1, q2_W2, q2_b2, q2_W3, q2_b3):
        eye = jnp.eye(NP, dtype=obs.dtype)

        def q_min(X):
            xi = jnp.concatenate([obs, X], axis=-1)
            return jnp.minimum(
                _q_net(xi, q1_W1, q1_b1, q1_W2, q1_b2, q1_W3, q1_b3),
                _q_net(xi, q2_W1, q2_b1, q2_W2, q2_b2, q2_W3, q2_b3))

        logq, vjp_fn = jax.vjp(q_min, a)
        score = vjp_fn(jnp.ones_like(logq))[0]
        X = a.reshape(B_loc, NP, ACT)
        S = score.reshape(B_loc, NP, ACT)
        diff = X[:, :, None, :] - X[:, None, :, :]
        dist_sq = (diff**2).sum(-1)
        K = jnp.exp(-GAMMA * dist_sq)
        K_grad = -2.0 * GAMMA * diff * K[..., None]
        phi = (jnp.einsum('bij,bjd->bid', K, S) + K_grad.sum(1)) / NP
        term1 = (K_grad * S[:, None, :, :]).sum(-1).mean(2)
        term2 = -2.0 * GAMMA * ((K_grad.transpose(0, 2, 1, 3) * diff).sum(-1)
                                - NP * (K - eye)).mean(1)
        logp = logp - LR * (term1 + term2)
        a = a + LR * phi.reshape(N_loc, ACT)
        a = jnp.clip(a, -ACT_LIMIT, ACT_LIMIT)
        return a, logp

    return step


def _finalize_host(a_pre, logp_svgd, a0):
    """tanh squash + log-prob assembly, float32 numpy on host."""
    a_pre = np.asarray(a_pre, dtype=np.float32).reshape(N, ACT)
    logp_svgd = np.asarray(logp_svgd, dtype=np.float32).reshape(B, NP)
    a = (ACT_LIMIT * np.tanh(a_pre)).astype(np.float32)
    logp_normal = (-ACT * 0.5 * np.log(2.0 * np.pi * SIGMA_P0)
                   - 0.5 / SIGMA_P0 * (a0**2).sum(-1).reshape(B, NP))
    # softplus(-2a), numerically fine for |a| <= 1
    sp = np.log1p(np.exp(-2.0 * a))
    logp_tanh = -(2.0 * (np.log(2.0) - a - sp)).sum(-1).reshape(B, NP)
    logp_a = (logp_normal + logp_svgd.astype(np.float64)
              + logp_tanh).mean(-1)
    return a, logp_a.astype(np.float32)


def _run_pmap(obs_sh, a0_sh):
    """Run the 10 SVGD steps data-parallel across 8 NeuronCores."""
    import jax

    B_loc = B // M_CORES
    devs = jax.devices()[:M_CORES]
    if 'step_fn' not in _cached:
        _cached['step_fn'] = jax.pmap(
            _make_step(B_loc), in_axes=(0,) * 15, devices=devs)
    step_fn = _cached['step_fn']

    # Ship inputs to the cores once; steps then only exchange device arrays.
    obs_dev = jax.device_put_sharded(list(obs_sh), devs)
    a = jax.device_put_sharded(list(a0_sh), devs)
    logp = jax.device_put_sharded(
        [np.zeros((B_loc, NP), dtype=np.float32)] * M_CORES, devs)
    wkey = _cached.get('wkey')
    if _cached.get('weights_dev') is None or wkey is not _cached['weights']:
        _cached['weights_dev'] = tuple(
            jax.device_put_replicated(w, devs) for w in _cached['weights'])
        _cached['wkey'] = _cached['weights']
    weights_dev = _cached['weights_dev']

    for _ in range(STEPS):
        a, logp = step_fn(a, logp, obs_dev, *weights_dev)
    a.block_until_ready()
    return np.asarray(a), np.asarray(logp)


def _run_host(obs, a0):
    import jax
    import jax.numpy as jnp

    cpu = jax.devices('cpu')[0]
    with jax.default_device(cpu):
        if 'cpu_fn' not in _cached:
            _cached['cpu_fn'] = jax.jit(_make_step(B))
        step_fn = _cached['cpu_fn']
        weights = _cached['weights']
        a = jnp.asarray(a0)
        logp = jnp.zeros((B, NP), dtype=jnp.float32)
        for _ in range(STEPS):
            a, logp = step_fn(a, logp, obs, *weights)
    return np.asarray(a), np.asarray(logp)


def kernel(obs, a0_table, q1_W1, q1_b1, q1_W2, q1_b2, q1_W3, q1_b3,
           q2_W1, q2_b1, q2_W2, q2_b2, q2_W3, q2_b3, idx):
    obs = np.asarray(obs, dtype=np.float32)
    a0 = np.ascontiguousarray(
        np.asarray(a0_table, dtype=np.float32)[np.asarray(idx)])
    _cached['weights'] = tuple(
        np.asarray(w, dtype=np.float32)
        for w in (q1_W1, q1_b1, q1_W2, q1_b2, q1_W3, q1_b3,
                  q2_W1, q2_b1, q2_W2, q2_b2, q2_W3, q2_b3))

    N_loc = N // M_CORES
    obs_sh = obs.reshape(M_CORES, N_loc, OBS)
    a0_sh = a0.reshape(M_CORES, N_loc, ACT)

    try:
        a_pre, logp_svgd = _run_pmap(obs_sh, a0_sh)
        if not (np.all(np.isfinite(a_pre))
                and np.all(np.isfinite(logp_svgd))):
            raise RuntimeError("non-finite device output")
    except Exception:
        _cached.pop('step_fn', None)
        a_pre, logp_svgd = _run_host(obs, a0)

    a, logp_a = _finalize_host(a_pre, logp_svgd, a0)
    return a.reshape(N, ACT), logp_a.reshape(B)


# revision 7
# speedup vs baseline: 33.3031x; 1.0311x over previous
"""SVGD actor sampler (nn_ActorSvgd) on 8 trn2 NeuronCores.

Strategy: pure data parallel over the env-batch dimension B (8192 rows of
NP=20 particles). Particles only interact within a batch row, so we:
  * gather a0 = a0_table[idx] on host (pure indexing),
  * shard obs/a0 on B across the 8 cores (1024 batch rows / 20480 particle
    rows per core), replicate the small Q-network weights,
  * run the SVGD loop as a host-driven loop of pmap'd single-step updates
    (one compiled step graph, executed 10x on all 8 cores; no collectives),
  * finalize (tanh + log-prob assembly) on host — tiny, and neuronxcc's
    lower_act pass rejects the tanh+softplus combination,
  * concatenate per-core outputs back to full shape.
Falls back to single-device CPU execution if the 8-core path is unavailable.
"""

import numpy as np

B = 8192
NP = 20
OBS = 17
ACT = 6
H = 256
N = B * NP
ACT_LIMIT = 1.0
SIGMA_P0 = 0.1
STEPS = 10
LR = 0.05
GAMMA = 1.0 / (1e-8 + 2.0 * 2.0**2)  # kernel_sigma = 2.0
M_CORES = 8

_cached = {}


def _make_step(B_loc, unroll=1):
    import jax
    import jax.numpy as jnp

    def _q_net(x, W1, b1, W2, b2, W3, b3):
        h = jax.nn.relu(x @ W1 + b1)
        h = jax.nn.relu(h @ W2 + b2)
        return (h @ W3 + b3)[:, 0]

    N_loc = B_loc * NP

    def step(a, logp, obs, q1_W1, q1_b1, q1_W2, q1_b2, q1_W3, q1_b3,
             q2_W1, q2_b1, q2_W2, q2_b2, q2_W3, q2_b3):
        eye = jnp.eye(NP, dtype=obs.dtype)

        def q_min(X):
            xi = jnp.concatenate([obs, X], axis=-1)
            return jnp.minimum(
                _q_net(xi, q1_W1, q1_b1, q1_W2, q1_b2, q1_W3, q1_b3),
                _q_net(xi, q2_W1, q2_b1, q2_W2, q2_b2, q2_W3, q2_b3))

        logq, vjp_fn = jax.vjp(q_min, a)
        score = vjp_fn(jnp.ones_like(logq))[0]
        X = a.reshape(B_loc, NP, ACT)
        S = score.reshape(B_loc, NP, ACT)
        diff = X[:, :, None, :] - X[:, None, :, :]
        dist_sq = (diff**2).sum(-1)
        K = jnp.exp(-GAMMA * dist_sq)
        K_grad = -2.0 * GAMMA * diff * K[..., None]
        phi = (jnp.einsum('bij,bjd->bid', K, S) + K_grad.sum(1)) / NP
        term1 = (K_grad * S[:, None, :, :]).sum(-1).mean(2)
        term2 = -2.0 * GAMMA * ((K_grad.transpose(0, 2, 1, 3) * diff).sum(-1)
                                - NP * (K - eye)).mean(1)
        logp = logp - LR * (term1 + term2)
        a = a + LR * phi.reshape(N_loc, ACT)
        a = jnp.clip(a, -ACT_LIMIT, ACT_LIMIT)
        return a, logp

    if unroll == 1:
        return step

    def multi(a, logp, obs, *w):
        for _ in range(unroll):
            a, logp = step(a, logp, obs, *w)
        return a, logp

    return multi


def _finalize_host(a_pre, logp_svgd, a0):
    """tanh squash + log-prob assembly, float32 numpy on host."""
    a_pre = np.asarray(a_pre, dtype=np.float32).reshape(N, ACT)
    logp_svgd = np.asarray(logp_svgd, dtype=np.float32).reshape(B, NP)
    a = (ACT_LIMIT * np.tanh(a_pre)).astype(np.float32)
    logp_normal = (-ACT * 0.5 * np.log(2.0 * np.pi * SIGMA_P0)
                   - 0.5 / SIGMA_P0 * (a0**2).sum(-1).reshape(B, NP))
    # softplus(-2a), numerically fine for |a| <= 1
    sp = np.log1p(np.exp(-2.0 * a))
    logp_tanh = -(2.0 * (np.log(2.0) - a - sp)).sum(-1).reshape(B, NP)
    logp_a = (logp_normal + logp_svgd.astype(np.float64)
              + logp_tanh).mean(-1)
    return a, logp_a.astype(np.float32)


def _run_pmap(obs_sh, a0_sh):
    """Run the 10 SVGD steps data-parallel across 8 NeuronCores."""
    import jax

    B_loc = B // M_CORES
    devs = jax.devices()[:M_CORES]

    # Ship inputs to the cores once; steps then only exchange device arrays.
    obs_dev = jax.device_put_sharded(list(obs_sh), devs)
    a0_dev = jax.device_put_sharded(list(a0_sh), devs)
    logp0 = jax.device_put_sharded(
        [np.zeros((B_loc, NP), dtype=np.float32)] * M_CORES, devs)
    wkey = _cached.get('wkey')
    if _cached.get('weights_dev') is None or wkey is not _cached['weights']:
        _cached['weights_dev'] = tuple(
            jax.device_put_replicated(w, devs) for w in _cached['weights'])
        _cached['wkey'] = _cached['weights']
    weights_dev = _cached['weights_dev']

    # Prefer fewer, larger compiled modules (fewer dispatch round-trips);
    # fall back to single-step modules if neuronxcc rejects the bigger graph.
    last_err = None
    for unroll in (2, 1):
        key = f'step_fn_{unroll}'
        try:
            if key not in _cached:
                _cached[key] = jax.pmap(
                    _make_step(B_loc, unroll), in_axes=(0,) * 15,
                    devices=devs)
            step_fn = _cached[key]
            a, logp = a0_dev, logp0
            for _ in range(STEPS // unroll):
                a, logp = step_fn(a, logp, obs_dev, *weights_dev)
            a.block_until_ready()
            return np.asarray(a), np.asarray(logp)
        except Exception as e:  # compile failure for this unroll level
            _cached.pop(key, None)
            last_err = e
    raise last_err


def _run_host(obs, a0):
    import jax
    import jax.numpy as jnp

    cpu = jax.devices('cpu')[0]
    with jax.default_device(cpu):
        if 'cpu_fn' not in _cached:
            _cached['cpu_fn'] = jax.jit(_make_step(B))
        step_fn = _cached['cpu_fn']
        weights = _cached['weights']
        a = jnp.asarray(a0)
        logp = jnp.zeros((B, NP), dtype=jnp.float32)
        for _ in range(STEPS):
            a, logp = step_fn(a, logp, obs, *weights)
    return np.asarray(a), np.asarray(logp)


def kernel(obs, a0_table, q1_W1, q1_b1, q1_W2, q1_b2, q1_W3, q1_b3,
           q2_W1, q2_b1, q2_W2, q2_b2, q2_W3, q2_b3, idx):
    obs = np.asarray(obs, dtype=np.float32)
    a0 = np.ascontiguousarray(
        np.asarray(a0_table, dtype=np.float32)[np.asarray(idx)])
    _cached['weights'] = tuple(
        np.asarray(w, dtype=np.float32)
        for w in (q1_W1, q1_b1, q1_W2, q1_b2, q1_W3, q1_b3,
                  q2_W1, q2_b1, q2_W2, q2_b2, q2_W3, q2_b3))

    N_loc = N // M_CORES
    obs_sh = obs.reshape(M_CORES, N_loc, OBS)
    a0_sh = a0.reshape(M_CORES, N_loc, ACT)

    try:
        a_pre, logp_svgd = _run_pmap(obs_sh, a0_sh)
        if not (np.all(np.isfinite(a_pre))
                and np.all(np.isfinite(logp_svgd))):
            raise RuntimeError("non-finite device output")
    except Exception:
        _cached.pop('step_fn', None)
        a_pre, logp_svgd = _run_host(obs, a0)

    a, logp_a = _finalize_host(a_pre, logp_svgd, a0)
    return a.reshape(N, ACT), logp_a.reshape(B)
